# revision 12
# baseline (speedup 1.0000x reference)
"""CovQuadraticCrossEntropyLoss Trainium2 kernel.

Reference computation, per (s, b) pair with V = 512:
    p    = softmax(m)                                  [V]
    quad = 0.5 * (sum_i K_ii p_i - p^T K p)
    ce   = logsumexp(m) - m[target]
    loss = ce + quad

Strategy (memory-bound: k is 512 MB total, 64 MB per core):
  - Fully data-parallel over the s axis: core i handles s in [4i, 4i+4),
    i.e. 64 (s, b) slabs of K [512, 512] each.
  - Per core, one fused softmax pass over m [64, 512] gives e = exp(m - max)
    and Z = sum(e) (p = e / Z is never materialized; the two p factors are
    divided out at the end: p^T K p = e^T K e / Z^2).
  - e is transposed once on the tensor engine to eT [128, 4, 64] so each
    slab's e can feed matmul as the stationary operand.
  - Per slab: DMA the 1 MB K tile as [128 part, 4 chunk, 512] (i = c*128+p),
    then 4 accumulating matmuls x[1, 512] += eT[:, c, s]^T @ K[:, c, :]
    compute x = K^T e while K streams at full rhs rate (N=512).
  - diag(K) for all slabs comes from one strided DMA (stride V+1 elements).
  - Epilogue is batched [64, *] vector work: dot products via
    tensor_tensor_reduce, the m[target] gather via an iota==target mask.
"""

import numpy as np

import concourse.bass as bass
import concourse.mybir as mybir
import concourse.tile as tile
from concourse.masks import make_identity

S, B, V = 32, 16, 512
N_CORES = 8
S_PER_CORE = S // N_CORES          # 4
SLABS = S_PER_CORE * B             # 64 (s, b) pairs per core
P = 128                            # partitions
CHUNKS = V // P                    # 4
F32 = mybir.dt.float32


def _split_multi_wait_instructions(nc: bass.Bass) -> None:
    """Rewrite the BIR so no instruction carries more than one sem wait.

    The walrus build here rejects instructions with >1 sync-wait command
    ("Too many sync wait commands", CoreV3GenImpl setupSyncWait). Engines
    execute their streams in order, so an instruction's extra waits can be
    moved onto same-engine NOPs inserted immediately before it.
    """
    for fn in nc.m.functions:
        for bb in fn.blocks:
            new_insts = []
            for inst in bb.instructions:
                si = inst.sync_info
                waits = list(si.on_wait) if si is not None and si.on_wait else []
                if len(waits) > 1:
                    for j, w in enumerate(waits[:-1]):
                        new_insts.append(
                            mybir.InstDrain(
                                name=f"{inst.name}-sw{j}",
                                engine=inst.engine,
                                bass_is_fusable=False,
                                sync_info=mybir.SyncInfo(on_wait=[w], on_update=[]),
                            )
                        )
                    inst.sync_info = mybir.SyncInfo(
                        on_wait=[waits[-1]],
                        on_update=list(si.on_update or []),
                    )
                new_insts.append(inst)
            bb.instructions = new_insts


def build_bass(k_bufs: int = 8, x_bufs: int = 6) -> bass.Bass:
    nc = bass.Bass(name="covq_ce")
    m_d = nc.dram_tensor("m", [SLABS, V], F32, kind="ExternalInput")
    k_d = nc.dram_tensor("k", [SLABS, V, V], F32, kind="ExternalInput")
    tgt_d = nc.dram_tensor("tgt", [SLABS, 1], F32, kind="ExternalInput")
    iota_d = nc.dram_tensor("iota", [SLABS, V], F32, kind="ExternalInput")
    out_d = nc.dram_tensor("out", [SLABS, 1], F32, kind="ExternalOutput")

    # K slab s as [partition p, chunk c, j] with row index i = c*128 + p.
    k_r = k_d[:, :, :].rearrange("n (c p) j -> n p c j", p=P)
    # diag(K) for every slab: element (n, i, i) = offset n*V*V + i*(V+1).
    diag_ap = bass.AP(tensor=k_d[:, :, :].tensor, offset=0, ap=[[V * V, SLABS], [V + 1, V]])

    with tile.TileContext(nc) as tc:
        with (
            tc.tile_pool(name="singles", bufs=1) as singles,
            tc.tile_pool(name="kpool", bufs=k_bufs) as kpool,
            tc.tile_pool(name="psum_t", bufs=1, space="PSUM") as psum_t,
            tc.tile_pool(name="psum_x", bufs=x_bufs, space="PSUM") as psum_x,
        ):
            # --- small input DMAs (SWDGE; HWDGE ring is reserved for K) ---
            m_sb = singles.tile([SLABS, V], F32)
            nc.gpsimd.dma_start(out=m_sb, in_=m_d[:, :])
            # 4-byte-granule gather: split to stay under the per-DMA
            # descriptor limit (each element is its own descriptor).
            diag_sb = singles.tile([SLABS, V], F32)
            import os as _os
            if _os.environ.get("KV", "") and "d" in _os.environ["KV"]:
                nc.vector.memset(diag_sb, 0.0)
            else:
                for q in range(4):
                    sl = slice(q * (SLABS // 4), (q + 1) * (SLABS // 4))
                    nc.gpsimd.dma_start(out=diag_sb[sl, :], in_=diag_ap[sl, :])
            iota_sb = singles.tile([SLABS, V], F32)
            nc.gpsimd.dma_start(out=iota_sb, in_=iota_d[:, :])
            tgt_sb = singles.tile([SLABS, 1], F32)
            nc.gpsimd.dma_start(out=tgt_sb, in_=tgt_d[:, :])

            identity = singles.tile([P, P], F32)
            make_identity(nc, identity)

            # --- softmax pieces: e = exp(m - max), Z = sum(e) --------------
            mx = singles.tile([SLABS, 1], F32)
            nc.vector.tensor_reduce(
                out=mx, in_=m_sb, axis=mybir.AxisListType.X, op=mybir.AluOpType.max
            )
            neg_mx = singles.tile([SLABS, 1], F32)
            nc.vector.tensor_scalar_mul(out=neg_mx, in0=mx, scalar1=-1.0)
            e_sb = singles.tile([SLABS, V], F32)
            z_sb = singles.tile([SLABS, 1], F32)
            KV = _os.environ.get("KV", "")
            if "A" in KV:
                nc.vector.memset(e_sb, 0.5)
                nc.vector.memset(z_sb, 1.0)
            elif "a" in KV:
                nc.scalar.activation(out=e_sb, in_=m_sb,
                    func=mybir.ActivationFunctionType.Exp, bias=neg_mx, scale=1.0)
                nc.vector.tensor_reduce(out=z_sb, in_=e_sb, axis=mybir.AxisListType.X, op=mybir.AluOpType.add)
            else:
                nc.scalar.activation(
                    out=e_sb,
                    in_=m_sb,
                    func=mybir.ActivationFunctionType.Exp,
                    bias=neg_mx,
                    scale=1.0,
                    accum_out=z_sb,
                )
            ln_z = singles.tile([SLABS, 1], F32)
            if "A" in KV:
                nc.vector.memset(ln_z, 0.0)
            else:
                nc.scalar.activation(out=ln_z, in_=z_sb, func=mybir.ActivationFunctionType.Ln)
            inv_z = singles.tile([SLABS, 1], F32)
            nc.vector.reciprocal(out=inv_z, in_=z_sb)

            # --- transpose e -> eT[p, c, s] so slab columns feed matmul ----
            eT_sb = singles.tile([P, CHUNKS, SLABS], F32)
            if "T" in KV:
                nc.vector.memset(eT_sb, 0.1)
            else:
                eT_ps = psum_t.tile([P, CHUNKS, SLABS], F32)
                for c in range(CHUNKS):
                    nc.tensor.transpose(
                        eT_ps[:, c, :],
                        e_sb[:, c * P : (c + 1) * P],
                        identity[:SLABS, :SLABS],
                    )
                nc.vector.tensor_copy(eT_sb, eT_ps)

            # --- main loop: stream K, x_s = K_s^T e_s ----------------------
            # Engine APs need 32-aligned partition bases, so x lands at
            # partition 0 (ACT copy out of PSUM) and a small SBUF->SBUF DMA
            # (no partition restriction) files it into row s of xs_sb.
            xs_sb = singles.tile([SLABS, V], F32)
            if ("x" in KV) or ("M" in KV):
                nc.vector.memset(xs_sb, 0.0)
            for s in range([] if "M" in KV else range(SLABS)) if False else (range(0) if "M" in KV else range(SLABS)):
                kt = kpool.tile([P, CHUNKS, V], F32, tag="kt")
                nc.gpsimd.dma_start(out=kt, in_=k_r[s])
                x_ps = psum_x.tile([1, V], F32, tag="x")
                for c in range(CHUNKS):
                    nc.tensor.matmul(
                        x_ps,
                        eT_sb[:, c, s : s + 1],
                        kt[:, c, :],
                        start=(c == 0),
                        stop=(c == CHUNKS - 1),
                    )
                x_row = kpool.tile([1, V], F32, tag="xrow")
                nc.scalar.copy(out=x_row, in_=x_ps)
                if not (_os.environ.get("KV", "") and "x" in _os.environ["KV"]):
                    nc.gpsimd.dma_start(out=xs_sb[s : s + 1, :], in_=x_row)

            # --- batched epilogue ------------------------------------------
            scratch = singles.tile([SLABS, V], F32, tag="scratch")
            msk = singles.tile([SLABS, V], F32)
            if "Q" in KV:
                nc.vector.memset(msk, 0.0)
            else:
                nc.vector.tensor_scalar(
                out=msk,
                in0=iota_sb,
                scalar1=tgt_sb,
                scalar2=None,
                    op0=mybir.AluOpType.is_equal,
                )
            g = singles.tile([SLABS, 1], F32)
            dq = singles.tile([SLABS, 1], F32)
            t_raw = singles.tile([SLABS, 1], F32)
            if "Q" in KV:
                nc.vector.memset(g, 0.0)
                nc.vector.memset(dq, 0.0)
                nc.vector.memset(t_raw, 0.0)
            else:
                nc.vector.tensor_mul(out=scratch, in0=msk, in1=m_sb)
                nc.vector.tensor_reduce(out=g, in_=scratch, axis=mybir.AxisListType.X, op=mybir.AluOpType.add)
                scratch2 = singles.tile([SLABS, V], F32, tag="scratch2")
                nc.vector.tensor_mul(out=scratch2, in0=diag_sb, in1=e_sb)
                nc.vector.tensor_reduce(out=dq, in_=scratch2, axis=mybir.AxisListType.X, op=mybir.AluOpType.add)
                scratch3 = singles.tile([SLABS, V], F32, tag="scratch3")
                nc.vector.tensor_mul(out=scratch3, in0=xs_sb, in1=e_sb)
                nc.vector.tensor_reduce(out=t_raw, in_=scratch3, axis=mybir.AxisListType.X, op=mybir.AluOpType.add)

            # loss = (mx + lnZ - g) + 0.5 * invZ * (dq - t_raw * invZ)
            t1 = singles.tile([SLABS, 1], F32)
            nc.vector.tensor_mul(out=t1, in0=t_raw, in1=inv_z)
            t2 = singles.tile([SLABS, 1], F32)
            nc.vector.tensor_sub(out=t2, in0=dq, in1=t1)
            t3 = singles.tile([SLABS, 1], F32)
            nc.vector.tensor_mul(out=t3, in0=t2, in1=inv_z)
            t4 = singles.tile([SLABS, 1], F32)
            nc.vector.tensor_scalar_mul(out=t4, in0=t3, scalar1=0.5)
            ce1 = singles.tile([SLABS, 1], F32)
            nc.vector.tensor_add(out=ce1, in0=mx, in1=ln_z)
            ce2 = singles.tile([SLABS, 1], F32)
            nc.vector.tensor_sub(out=ce2, in0=ce1, in1=g)
            loss = singles.tile([SLABS, 1], F32)
            nc.vector.tensor_add(out=loss, in0=ce2, in1=t4)

            nc.gpsimd.dma_start(out=out_d[:, :], in_=loss)

    _split_multi_wait_instructions(nc)
    return nc


_NC_CACHE = {}


def _get_nc():
    if "nc" not in _NC_CACHE:
        _NC_CACHE["nc"] = build_bass()
    return _NC_CACHE["nc"]


_IOTA = np.broadcast_to(np.arange(V, dtype=np.float32), (SLABS, V)).copy()


def run_sharded(m, k, target, trace=False, **run_kwargs):
    """Shard full inputs over 8 cores, run the bass kernel, gather output.

    Returns (loss [S, B] f32, BassKernelResults).
    """
    from concourse.bass_utils import run_bass_kernel_spmd

    m = np.ascontiguousarray(np.asarray(m), dtype=np.float32)
    k = np.ascontiguousarray(np.asarray(k), dtype=np.float32)
    target = np.asarray(target)
    assert m.shape == (S, B, V) and k.shape == (S, B, V, V)
    tgt_f = target.astype(np.float32).reshape(S, B)

    in_maps = []
    for c in range(N_CORES):
        sl = slice(c * S_PER_CORE, (c + 1) * S_PER_CORE)
        in_maps.append(
            {
                "m": m[sl].reshape(SLABS, V),
                "k": k[sl].reshape(SLABS, V, V),
                "tgt": tgt_f[sl].reshape(SLABS, 1),
                "iota": _IOTA,
            }
        )

    res = run_bass_kernel_spmd(
        _get_nc(), in_maps, core_ids=list(range(N_CORES)), trace=trace, **run_kwargs
    )
    loss = np.concatenate(
        [r["out"].reshape(S_PER_CORE, B) for r in res.results], axis=0
    )
    return loss, res


def kernel(m, k, target):
    loss, _ = run_sharded(m, k, target)
    return loss


# revision 13
# speedup vs baseline: 1.6682x; 1.6682x over previous
"""CovQuadraticCrossEntropyLoss Trainium2 kernel.

Reference computation, per (s, b) pair with V = 512:
    p    = softmax(m)                                  [V]
    quad = 0.5 * (sum_i K_ii p_i - p^T K p)
    ce   = logsumexp(m) - m[target]
    loss = ce + quad

Strategy (memory-bound: k is 512 MB total, 64 MB per core):
  - Fully data-parallel over the s axis: core i handles s in [4i, 4i+4),
    i.e. 64 (s, b) slabs of K [512, 512] each.
  - Per core, one fused softmax pass over m [64, 512] gives e = exp(m - max)
    and Z = sum(e) (p = e / Z is never materialized; the two p factors are
    divided out at the end: p^T K p = e^T K e / Z^2).
  - e is transposed once on the tensor engine to eT [128, 4, 64] so each
    slab's e can feed matmul as the stationary operand.
  - Per slab: DMA the 1 MB K tile as [128 part, 4 chunk, 512] (i = c*128+p),
    then 4 accumulating matmuls x[1, 512] += eT[:, c, s]^T @ K[:, c, :]
    compute x = K^T e while K streams at full rhs rate (N=512).
  - diag(K) for all slabs comes from one strided DMA (stride V+1 elements).
  - Epilogue is batched [64, *] vector work: dot products via
    tensor_tensor_reduce, the m[target] gather via an iota==target mask.
"""

import numpy as np

import concourse.bass as bass
import concourse.mybir as mybir
import concourse.tile as tile
from concourse.masks import make_identity

S, B, V = 32, 16, 512
N_CORES = 8
S_PER_CORE = S // N_CORES          # 4
SLABS = S_PER_CORE * B             # 64 (s, b) pairs per core
P = 128                            # partitions
CHUNKS = V // P                    # 4
F32 = mybir.dt.float32


def _split_multi_wait_instructions(nc: bass.Bass) -> None:
    """Rewrite the BIR so no instruction carries more than one sem wait.

    The walrus build here rejects instructions with >1 sync-wait command
    ("Too many sync wait commands", CoreV3GenImpl setupSyncWait). Engines
    execute their streams in order, so an instruction's extra waits can be
    moved onto same-engine NOPs inserted immediately before it.
    """
    for fn in nc.m.functions:
        for bb in fn.blocks:
            new_insts = []
            for inst in bb.instructions:
                si = inst.sync_info
                waits = list(si.on_wait) if si is not None and si.on_wait else []
                if len(waits) > 1:
                    for j, w in enumerate(waits[:-1]):
                        new_insts.append(
                            mybir.InstDrain(
                                name=f"{inst.name}-sw{j}",
                                engine=inst.engine,
                                bass_is_fusable=False,
                                sync_info=mybir.SyncInfo(on_wait=[w], on_update=[]),
                            )
                        )
                    inst.sync_info = mybir.SyncInfo(
                        on_wait=[waits[-1]],
                        on_update=list(si.on_update or []),
                    )
                new_insts.append(inst)
            bb.instructions = new_insts


def build_bass(k_bufs: int = 8, x_bufs: int = 6) -> bass.Bass:
    nc = bass.Bass(name="covq_ce")
    m_d = nc.dram_tensor("m", [SLABS, V], F32, kind="ExternalInput")
    k_d = nc.dram_tensor("k", [SLABS, V, V], F32, kind="ExternalInput")
    tgt_d = nc.dram_tensor("tgt", [SLABS, 1], F32, kind="ExternalInput")
    iota_d = nc.dram_tensor("iota", [SLABS, V], F32, kind="ExternalInput")
    out_d = nc.dram_tensor("out", [SLABS, 1], F32, kind="ExternalOutput")

    # K slab s as [partition p, chunk c, j] with row index i = c*128 + p.
    k_r = k_d[:, :, :].rearrange("n (c p) j -> n p c j", p=P)
    # diag(K) for every slab: element (n, i, i) = offset n*V*V + i*(V+1).
    diag_ap = bass.AP(tensor=k_d[:, :, :].tensor, offset=0, ap=[[V * V, SLABS], [V + 1, V]])

    with tile.TileContext(nc) as tc:
        with (
            tc.tile_pool(name="singles", bufs=1) as singles,
            tc.tile_pool(name="kpool", bufs=k_bufs) as kpool,
            tc.tile_pool(name="psum_t", bufs=1, space="PSUM") as psum_t,
            tc.tile_pool(name="psum_x", bufs=x_bufs, space="PSUM") as psum_x,
        ):
            # --- small input DMAs (SWDGE; HWDGE ring is reserved for K) ---
            m_sb = singles.tile([SLABS, V], F32)
            nc.gpsimd.dma_start(out=m_sb, in_=m_d[:, :])
            # 4-byte-granule gather: split to stay under the per-DMA
            # descriptor limit (each element is its own descriptor).
            diag_sb = singles.tile([SLABS, V], F32)
            import os as _os
            if _os.environ.get("KV", "") and "d" in _os.environ["KV"]:
                nc.vector.memset(diag_sb, 0.0)
            else:
                for q in range(4):
                    sl = slice(q * (SLABS // 4), (q + 1) * (SLABS // 4))
                    nc.gpsimd.dma_start(out=diag_sb[sl, :], in_=diag_ap[sl, :])
            iota_sb = singles.tile([SLABS, V], F32)
            nc.gpsimd.dma_start(out=iota_sb, in_=iota_d[:, :])
            tgt_sb = singles.tile([SLABS, 1], F32)
            nc.gpsimd.dma_start(out=tgt_sb, in_=tgt_d[:, :])

            identity = singles.tile([P, P], F32)
            make_identity(nc, identity)

            # --- softmax pieces: e = exp(m - max), Z = sum(e) --------------
            mx = singles.tile([SLABS, 1], F32)
            nc.vector.tensor_reduce(
                out=mx, in_=m_sb, axis=mybir.AxisListType.X, op=mybir.AluOpType.max
            )
            neg_mx = singles.tile([SLABS, 1], F32)
            nc.vector.tensor_scalar_mul(out=neg_mx, in0=mx, scalar1=-1.0)
            e_sb = singles.tile([SLABS, V], F32)
            z_sb = singles.tile([SLABS, 1], F32)
            KV = _os.environ.get("KV", "")
            if "A" in KV:
                nc.vector.memset(e_sb, 0.5)
                nc.vector.memset(z_sb, 1.0)
            elif "a" in KV:
                nc.scalar.activation(out=e_sb, in_=m_sb,
                    func=mybir.ActivationFunctionType.Exp, bias=neg_mx, scale=1.0)
                nc.vector.tensor_reduce(out=z_sb, in_=e_sb, axis=mybir.AxisListType.X, op=mybir.AluOpType.add)
            else:
                nc.scalar.activation(
                    out=e_sb,
                    in_=m_sb,
                    func=mybir.ActivationFunctionType.Exp,
                    bias=neg_mx,
                    scale=1.0,
                    accum_out=z_sb,
                )
            ln_z = singles.tile([SLABS, 1], F32)
            if "A" in KV:
                nc.vector.memset(ln_z, 0.0)
            else:
                nc.scalar.activation(out=ln_z, in_=z_sb, func=mybir.ActivationFunctionType.Ln)
            inv_z = singles.tile([SLABS, 1], F32)
            nc.vector.reciprocal(out=inv_z, in_=z_sb)

            # --- transpose e -> eT[p, c, s] so slab columns feed matmul ----
            eT_sb = singles.tile([P, CHUNKS, SLABS], F32)
            if "T" in KV:
                nc.vector.memset(eT_sb, 0.1)
            else:
                eT_ps = psum_t.tile([P, CHUNKS, SLABS], F32)
                for c in range(CHUNKS):
                    nc.tensor.transpose(
                        eT_ps[:, c, :],
                        e_sb[:, c * P : (c + 1) * P],
                        identity[:SLABS, :SLABS],
                    )
                nc.vector.tensor_copy(eT_sb, eT_ps)

            # --- main loop: stream K, x_s = K_s^T e_s ----------------------
            # Engine APs need 32-aligned partition bases, so x lands at
            # partition 0 (ACT copy out of PSUM) and a small SBUF->SBUF DMA
            # (no partition restriction) files it into row s of xs_sb.
            xs_sb = singles.tile([SLABS, V], F32)
            if ("x" in KV) or ("M" in KV):
                nc.vector.memset(xs_sb, 0.0)
            for s in range([] if "M" in KV else range(SLABS)) if False else (range(0) if "M" in KV else range(SLABS)):
                kt = kpool.tile([P, CHUNKS, V], F32, tag="kt")
                nc.sync.dma_start(out=kt, in_=k_r[s])
                x_ps = psum_x.tile([1, V], F32, tag="x")
                for c in range(CHUNKS):
                    nc.tensor.matmul(
                        x_ps,
                        eT_sb[:, c, s : s + 1],
                        kt[:, c, :],
                        start=(c == 0),
                        stop=(c == CHUNKS - 1),
                    )
                x_row = kpool.tile([1, V], F32, tag="xrow")
                nc.scalar.copy(out=x_row, in_=x_ps)
                if not (_os.environ.get("KV", "") and "x" in _os.environ["KV"]):
                    nc.gpsimd.dma_start(out=xs_sb[s : s + 1, :], in_=x_row)

            # --- batched epilogue ------------------------------------------
            scratch = singles.tile([SLABS, V], F32, tag="scratch")
            msk = singles.tile([SLABS, V], F32)
            if "Q" in KV:
                nc.vector.memset(msk, 0.0)
            else:
                nc.vector.tensor_scalar(
                out=msk,
                in0=iota_sb,
                scalar1=tgt_sb,
                scalar2=None,
                    op0=mybir.AluOpType.is_equal,
                )
            g = singles.tile([SLABS, 1], F32)
            dq = singles.tile([SLABS, 1], F32)
            t_raw = singles.tile([SLABS, 1], F32)
            if "Q" in KV:
                nc.vector.memset(g, 0.0)
                nc.vector.memset(dq, 0.0)
                nc.vector.memset(t_raw, 0.0)
            else:
                nc.vector.tensor_mul(out=scratch, in0=msk, in1=m_sb)
                nc.vector.tensor_reduce(out=g, in_=scratch, axis=mybir.AxisListType.X, op=mybir.AluOpType.add)
                scratch2 = singles.tile([SLABS, V], F32, tag="scratch2")
                nc.vector.tensor_mul(out=scratch2, in0=diag_sb, in1=e_sb)
                nc.vector.tensor_reduce(out=dq, in_=scratch2, axis=mybir.AxisListType.X, op=mybir.AluOpType.add)
                scratch3 = singles.tile([SLABS, V], F32, tag="scratch3")
                nc.vector.tensor_mul(out=scratch3, in0=xs_sb, in1=e_sb)
                nc.vector.tensor_reduce(out=t_raw, in_=scratch3, axis=mybir.AxisListType.X, op=mybir.AluOpType.add)

            # loss = (mx + lnZ - g) + 0.5 * invZ * (dq - t_raw * invZ)
            t1 = singles.tile([SLABS, 1], F32)
            nc.vector.tensor_mul(out=t1, in0=t_raw, in1=inv_z)
            t2 = singles.tile([SLABS, 1], F32)
            nc.vector.tensor_sub(out=t2, in0=dq, in1=t1)
            t3 = singles.tile([SLABS, 1], F32)
            nc.vector.tensor_mul(out=t3, in0=t2, in1=inv_z)
            t4 = singles.tile([SLABS, 1], F32)
            nc.vector.tensor_scalar_mul(out=t4, in0=t3, scalar1=0.5)
            ce1 = singles.tile([SLABS, 1], F32)
            nc.vector.tensor_add(out=ce1, in0=mx, in1=ln_z)
            ce2 = singles.tile([SLABS, 1], F32)
            nc.vector.tensor_sub(out=ce2, in0=ce1, in1=g)
            loss = singles.tile([SLABS, 1], F32)
            nc.vector.tensor_add(out=loss, in0=ce2, in1=t4)

            nc.gpsimd.dma_start(out=out_d[:, :], in_=loss)

    _split_multi_wait_instructions(nc)
    return nc


_NC_CACHE = {}


def _get_nc():
    if "nc" not in _NC_CACHE:
        _NC_CACHE["nc"] = build_bass()
    return _NC_CACHE["nc"]


_IOTA = np.broadcast_to(np.arange(V, dtype=np.float32), (SLABS, V)).copy()


def run_sharded(m, k, target, trace=False, **run_kwargs):
    """Shard full inputs over 8 cores, run the bass kernel, gather output.

    Returns (loss [S, B] f32, BassKernelResults).
    """
    from concourse.bass_utils import run_bass_kernel_spmd

    m = np.ascontiguousarray(np.asarray(m), dtype=np.float32)
    k = np.ascontiguousarray(np.asarray(k), dtype=np.float32)
    target = np.asarray(target)
    assert m.shape == (S, B, V) and k.shape == (S, B, V, V)
    tgt_f = target.astype(np.float32).reshape(S, B)

    in_maps = []
    for c in range(N_CORES):
        sl = slice(c * S_PER_CORE, (c + 1) * S_PER_CORE)
        in_maps.append(
            {
                "m": m[sl].reshape(SLABS, V),
                "k": k[sl].reshape(SLABS, V, V),
                "tgt": tgt_f[sl].reshape(SLABS, 1),
                "iota": _IOTA,
            }
        )

    res = run_bass_kernel_spmd(
        _get_nc(), in_maps, core_ids=list(range(N_CORES)), trace=trace, **run_kwargs
    )
    loss = np.concatenate(
        [r["out"].reshape(S_PER_CORE, B) for r in res.results], axis=0
    )
    return loss, res


def kernel(m, k, target):
    loss, _ = run_sharded(m, k, target)
    return loss
